# revision 9
# baseline (speedup 1.0000x reference)
"""Causal self-attention (B=4, S=2048, D=1024, single head) on 8 TRN2 cores.

Sharding: core c handles batch b = c//2 with query-tile parity p = c%2 —
its 8 query tiles of 128 rows are the absolute 128-row tiles {2j+p}.
Interleaving parities balances causal work exactly; the single SPMD
program is uniform and per-core variation is data-only (query rows and
the additive causal mask for the last kv group).

Re-associated algebra removes the k and v projections entirely:
  scores = q @ k.T = ((x_q Wq^T) Wk) @ x^T      (only q-rows projected)
  out    = attn @ v = (attn @ x) @ Wv^T          (project the context)

All matmul operands are bf16 (fp32 PSUM accumulation). Every DRAM input
is pre-arranged host-side into its exact SBUF flat layout, so each DMA
is a contiguous [128, cols] block copy ordered to match consumption.
The PE is pre-warmed with dummy matmuls during the initial DMA wait
(post-idle it runs at reduced p-state for ~3us).

PE program order (software-pipelined):
  A: qT[o,sq] = wqT.T @ xqT          B: qkT[d,sq] = wk.T @ qT
  S(0) Ptr(0) ctx(0) | S(1) | for j>=1: ctxT(j-1) Ptr(j) out(j-1)
  ctx(j) S(j+1) | ctxT(7) out(7)
where S = scores+exp (fused row-sum), Ptr = PE-transpose of P,
ctx = PT.T @ xnat (then *1/rowsum, cast bf16), ctxT = PE-transpose,
out = ctxT.T @ wvT.
"""

import numpy as np
import ml_dtypes

B, S, D = 4, 2048, 1024
DC = D // 128          # 128-chunks along d / o
NB = S // 128          # kv blocks per batch
NT = 8                 # q tiles per core
SCALE = 1.0 / np.sqrt(np.float32(D))
NEG = np.float32(-1e30)
BF16 = ml_dtypes.bfloat16

_cache = {}


def _ext(j):
    # kv extent for local tile j in 128-blocks (uniform across cores);
    # rounded up to a multiple of 2 so the tail group is 256-wide
    return 2 * j + 2


def _build():
    if "nc" in _cache:
        return _cache["nc"]

    import concourse.bacc as bacc
    import concourse.mybir as mybir
    import concourse.tile as tile

    f32 = mybir.dt.float32
    bf16 = mybir.dt.bfloat16
    AF = mybir.ActivationFunctionType

    nc = bacc.Bacc("TRN2", target_bir_lowering=False, debug=False,
                   num_devices=8)
    # all inputs pre-arranged host-side to the SBUF layout, [128, cols]
    xq_d = nc.dram_tensor("xq", [128, 2 * DC * 512], bf16,
                          kind="ExternalInput").ap()    # [p, sg,dc,s]
    xkv_d = nc.dram_tensor("xkv", [128, DC * S], bf16,
                           kind="ExternalInput").ap()   # [p, dc,kv]
    xn_d = nc.dram_tensor("xn", [128, NB * D], bf16,
                          kind="ExternalInput").ap()    # [p, kb,d]
    wq_d = nc.dram_tensor("wq", [128, DC * D], bf16,
                          kind="ExternalInput").ap()    # [p, ot,dc,oo]
    wk_d = nc.dram_tensor("wk", [128, DC * D], bf16,
                          kind="ExternalInput").ap()    # [p, dc,oc,dd]
    wv_d = nc.dram_tensor("wv", [128, DC * D], bf16,
                          kind="ExternalInput").ap()    # [p, dc,o]
    masks_d = nc.dram_tensor("masks", [128, NT * 512], bf16,
                             kind="ExternalInput").ap()  # [p, j,k]
    ident_d = nc.dram_tensor("ident", [128, 128], bf16,
                             kind="ExternalInput").ap()
    out_d = nc.dram_tensor("out", [NT * 128, D], f32,
                           kind="ExternalOutput").ap()

    with tile.TileContext(nc) as tc:
        with (
            tc.tile_pool(name="persist", bufs=1) as persist,
            tc.tile_pool(name="wpool", bufs=2) as wp,
            tc.tile_pool(name="pD", bufs=2) as pp,
            tc.tile_pool(name="ptD", bufs=1) as ptp,
            tc.tile_pool(name="cD", bufs=2) as cp,
            tc.tile_pool(name="ctD", bufs=2) as ctp,
            tc.tile_pool(name="oD", bufs=2) as op,
            tc.tile_pool(name="smD", bufs=2) as smp,
        ):
            xkvT = persist.tile([128, DC * S], bf16)
            xnat = persist.tile([128, NB * D], bf16)
            qT = persist.tile([128, DC * NT * 128], bf16)
            qkT = persist.tile([128, DC * NT * 128], bf16)
            masks = persist.tile([128, NT * 512], bf16)
            ident = persist.tile([128, 128], bf16)
            warm = persist.tile([128, 1], f32)
            scr = persist.tile([128, 512], bf16)

            def load(dst, src, cuts, eng):
                for a, b in zip(cuts[:-1], cuts[1:]):
                    eng.dma_start(dst[:, a:b], src[:, a:b])

            Ph, rch, cth = {}, {}, {}

            def scores_exp(j, pool):
                ext = _ext(j)
                ng = (ext + 3) // 4
                P = pp.tile([128, NB * 128], bf16, tag="P", name=f"P{j}")
                dslots = smp.tile([128, 4], f32, tag="ds", name=f"ds{j}")
                for g in range(ng):
                    gw = min(512, ext * 128 - g * 512)
                    last = (g == ng - 1)
                    sps = pool.tile([128, 512], f32, tag="sc",
                                    name=f"sps{j}_{g}")
                    for dc in range(DC):
                        nc.tensor.matmul(
                            sps[:, 0:gw],
                            qkT[:, dc * 1024 + j * 128:dc * 1024 + j * 128 + 128],
                            xkvT[:, dc * S + g * 512:dc * S + g * 512 + gw],
                            start=(dc == 0),
                            stop=(dc == DC - 1 and not last))
                    if last:
                        nc.tensor.matmul(
                            sps[:, 0:gw], ident[:],
                            masks[:, j * 512:j * 512 + gw],
                            start=False, stop=True)
                    nc.scalar.activation(
                        P[:, g * 512:g * 512 + gw], sps[:, 0:gw], AF.Exp,
                        scale=float(SCALE),
                        accum_out=dslots[:, g:g + 1])
                rcp = smp.tile([128, 1], f32, tag="rcp", name=f"rcp{j}")
                den = smp.tile([128, 1], f32, tag="den", name=f"den{j}")
                nc.vector.reduce_sum(den[:], dslots[:, 0:ng],
                                     axis=mybir.AxisListType.X)
                nc.vector.reciprocal(rcp[:], den[:])
                Ph[j], rch[j] = P, rcp

            with (
                tc.tile_pool(name="xqpool", bufs=1) as xqs,
                tc.tile_pool(name="psAB", bufs=4, space="PSUM") as psA,
            ):
                # warm-up: Exp table load + PE p-state ramp during DMA wait
                nc.gpsimd.memset(warm[:], 0.0)
                nc.scalar.activation(warm[:], warm[:], AF.Exp)
                nc.gpsimd.memset(scr[:], 0.0)

                # ---- input DMA: contiguous pieces, first-needed first ----
                xq_sb = xqs.tile([128, 2 * DC * 512], bf16, tag="xq")
                wq = wp.tile([128, DC * D], bf16, name="w_wq", tag="w")
                wk = wp.tile([128, DC * D], bf16, name="w_wk", tag="w")
                K = 1024
                load(wq[:], wq_d, (0, 2 * K, 4 * K, 6 * K, 8 * K), nc.sync)
                load(xq_sb[:], xq_d, (0, 2 * K, 4 * K, 6 * K, 8 * K),
                     nc.scalar)
                nc.scalar.dma_start(ident[:], ident_d)
                load(wk[:], wk_d, (0, 4 * K, 8 * K), nc.sync)
                nc.scalar.dma_start(masks[:], masks_d)
                load(xkvT[:], xkv_d, (0, 4 * K, 8 * K, 12 * K, 16 * K),
                     nc.sync)
                wv = wp.tile([128, DC * D], bf16, name="w_wv", tag="w")
                load(wv[:], wv_d, (0, 4 * K, 8 * K), nc.sync)
                load(xnat[:], xn_d, (0, 4 * K, 8 * K, 12 * K, 16 * K),
                     nc.scalar)

                # PE p-state warm-up: dummy matmuls on memset scratch,
                # sized to bridge until the first wq/xq pieces land
                for _ in range(16):
                    ps = psA.tile([128, 512], f32, tag="pj")
                    nc.tensor.matmul(ps[:], scr[:, 0:128], scr[:],
                                     start=True, stop=True)

                # ---- Phase A: q projection (into resident qT) ----
                for sg in range(2):
                    for ot in range(8):
                        ps = psA.tile([128, 512], f32, tag="pj")
                        for dc in range(DC):
                            nc.tensor.matmul(
                                ps[:],
                                wq[:, ot * 1024 + dc * 128:ot * 1024 + dc * 128 + 128],
                                xq_sb[:, sg * 4096 + dc * 512:sg * 4096 + dc * 512 + 512],
                                start=(dc == 0), stop=(dc == DC - 1))
                        nc.vector.tensor_copy(
                            qT[:, ot * 1024 + sg * 512:ot * 1024 + sg * 512 + 512],
                            ps[:])

                # ---- Phase B: fold Wk into q (qk = q @ Wk, transposed) ----
                # scores(0)/(1) are slotted into the B window so exp(0/1)
                # latency hides under B's matmuls
                for sg in range(2):
                    for dc in range(DC):
                        ps = psA.tile([128, 512], f32, tag="pj")
                        for oc in range(DC):
                            nc.tensor.matmul(
                                ps[:],
                                wk[:, dc * 1024 + oc * 128:dc * 1024 + oc * 128 + 128],
                                qT[:, oc * 1024 + sg * 512:oc * 1024 + sg * 512 + 512],
                                start=(oc == 0), stop=(oc == DC - 1))
                        nc.scalar.copy(
                            qkT[:, dc * 1024 + sg * 512:dc * 1024 + sg * 512 + 512],
                            ps[:])
                    scores_exp(sg, psA)

            # ---- attention ----
            with (
                tc.tile_pool(name="psS", bufs=2, space="PSUM") as ps_s,
                tc.tile_pool(name="psT", bufs=2, space="PSUM") as ps_t,
                tc.tile_pool(name="psM", bufs=4, space="PSUM") as ps_m,
            ):
                def ptr(j):
                    ext = _ext(j)
                    P = Ph[j]
                    PT = ptp.tile([128, NB * 128], bf16, tag="PT",
                                  name=f"PT{j}")
                    for g in range((ext + 3) // 4):
                        nb = min(4, ext - g * 4)
                        tps = ps_t.tile([128, 512], bf16, tag="tp",
                                        name=f"tps{j}_{g}")
                        for bb in range(nb):
                            nc.tensor.transpose(
                                tps[:, bb * 128:(bb + 1) * 128],
                                P[:, g * 512 + bb * 128:g * 512 + bb * 128 + 128],
                                ident[:])
                        nc.vector.tensor_copy(
                            PT[:, g * 512:g * 512 + nb * 128],
                            tps[:, 0:nb * 128])
                    return PT

                def ctx_mm(j, PT):
                    ext = _ext(j)
                    ctx = cp.tile([128, D], bf16, tag="ctx", name=f"ctx{j}")
                    for og in range(2):
                        ops = ps_m.tile([128, 512], f32, tag="av",
                                        name=f"av{j}_{og}")
                        for kb in range(ext):
                            nc.tensor.matmul(
                                ops[:],
                                PT[:, kb * 128:(kb + 1) * 128],
                                xnat[:, kb * D + og * 512:kb * D + og * 512 + 512],
                                start=(kb == 0), stop=(kb == ext - 1))
                        # normalize+cast on ScalarE to keep VectorE free for
                        # the PT/ctxT copies that gate the PE
                        nc.scalar.mul(
                            ctx[:, og * 512:(og + 1) * 512], ops[:],
                            rch[j][:])
                    cth[j] = ctx

                def ctxT_tr(j):
                    ctx = cth[j]
                    ctxT = ctp.tile([128, D], bf16, tag="ctxT",
                                    name=f"ctxT{j}")
                    for h in range(2):
                        tps = ps_t.tile([128, 512], bf16, tag="tp",
                                        name=f"tpc{j}_{h}")
                        for q4 in range(4):
                            dc = h * 4 + q4
                            nc.tensor.transpose(
                                tps[:, q4 * 128:(q4 + 1) * 128],
                                ctx[:, dc * 128:dc * 128 + 128],
                                ident[:])
                        nc.vector.tensor_copy(
                            ctxT[:, h * 512:(h + 1) * 512], tps[:])
                    return ctxT

                def out_mm(j, ctxT):
                    osb = op.tile([128, D], f32, tag="o", name=f"o{j}")
                    out3 = out_d.rearrange("q (og o) -> q og o", og=2)
                    for og in range(2):
                        ops = ps_m.tile([128, 512], f32, tag="av",
                                        name=f"op{j}_{og}")
                        for dc in range(DC):
                            nc.tensor.matmul(
                                ops[:],
                                ctxT[:, dc * 128:dc * 128 + 128],
                                wv[:, dc * 1024 + og * 512:dc * 1024 + og * 512 + 512],
                                start=(dc == 0), stop=(dc == DC - 1))
                        nc.vector.tensor_copy(
                            osb[:, og * 512:(og + 1) * 512], ops[:])
                        nc.sync.dma_start(
                            out3[j * 128:(j + 1) * 128, og],
                            osb[:, og * 512:(og + 1) * 512])

                # software pipeline (S(0)/S(1) already issued in B window):
                #   Ptr(0) ctx(0) | j: ctxT(j-1) Ptr(j) out(j-1) ctx(j)
                #   S(j+1) | ctxT(7) out(7)
                PT = ptr(0)
                ctx_mm(0, PT)
                for j in range(1, NT):
                    cT = ctxT_tr(j - 1)
                    PT = ptr(j)
                    out_mm(j - 1, cT)
                    ctx_mm(j, PT)
                    if j + 1 < NT:
                        scores_exp(j + 1, ps_s)
                cT = ctxT_tr(NT - 1)
                out_mm(NT - 1, cT)

    nc.compile()
    _cache["nc"] = nc
    return nc


def _shard(x, Wq, Wk, Wv):
    """Build the 8 per-core input maps, pre-arranged to SBUF layouts."""
    ident = np.eye(128, dtype=np.float32).astype(BF16)
    # wq2[p, ot*1024+dc*128+oo] = Wq[ot*128+oo, dc*128+p]
    wq2 = np.ascontiguousarray(
        Wq.reshape(8, 128, 8, 128).transpose(3, 0, 2, 1)
        .reshape(128, 8192)).astype(BF16)
    # wk2[p, dc*1024+oc*128+dd] = Wk[oc*128+p, dc*128+dd]
    wk2 = np.ascontiguousarray(
        Wk.reshape(8, 128, 8, 128).transpose(1, 2, 0, 3)
        .reshape(128, 8192)).astype(BF16)
    # wv2[p, dc*1024+o] = Wv[o, dc*128+p]
    wv2 = np.ascontiguousarray(
        Wv.T.reshape(8, 128, 1024).transpose(1, 0, 2)
        .reshape(128, 8192)).astype(BF16)
    in_maps = []
    for c in range(8):
        b, p = c // 2, c % 2
        xb = np.asarray(x[b])
        # xkv2[p, dc*2048+kv] = x[kv, dc*128+p]
        xkv2 = np.ascontiguousarray(
            xb.T.reshape(8, 128, 2048).transpose(1, 0, 2)
            .reshape(128, 16384)).astype(BF16)
        # xn2[p, kb*1024+d] = x[kb*128+p, d]
        xn2 = np.ascontiguousarray(
            xb.reshape(16, 128, 1024).transpose(1, 0, 2)
            .reshape(128, 16384)).astype(BF16)
        rows = np.concatenate(
            [xb[(2 * j + p) * 128:(2 * j + p + 1) * 128, :] for j in range(8)],
            axis=0)
        # xq2[p, sg*4096+dc*512+ss] = rows[sg*512+ss, dc*128+p]
        xq2 = np.ascontiguousarray(
            rows.reshape(2, 512, 8, 128).transpose(3, 0, 2, 1)
            .reshape(128, 8192)).astype(BF16)
        masks = np.full((NT * 128, 512), NEG, np.float32)
        for j in range(NT):
            ext = _ext(j)
            ng = (ext + 3) // 4
            gw = min(512, ext * 128 - (ng - 1) * 512)
            q_abs = (2 * j + p) * 128 + np.arange(128)[:, None]
            kv_abs = (ng - 1) * 512 + np.arange(gw)[None, :]
            masks[j * 128:(j + 1) * 128, 0:gw] = np.where(
                kv_abs <= q_abs, np.float32(0), NEG)
        masks2 = np.ascontiguousarray(
            masks.reshape(8, 128, 512).transpose(1, 0, 2)
            .reshape(128, 4096)).astype(BF16)
        in_maps.append({
            "xq": xq2, "xkv": xkv2, "xn": xn2,
            "wq": wq2, "wk": wk2, "wv": wv2,
            "masks": masks2, "ident": ident,
        })
    return in_maps


def _unshard(results, dtype):
    out = np.empty((B, S, D), dtype)
    for c in range(8):
        b, p = c // 2, c % 2
        o = results[c]["out"]
        for j in range(NT):
            out[b, (2 * j + p) * 128:(2 * j + p + 1) * 128, :] = \
                o[j * 128:(j + 1) * 128, :]
    return out


def run(x, Wq, Wk, Wv, trace=False):
    from concourse.bass_utils import run_bass_kernel_spmd
    nc = _build()
    in_maps = _shard(np.asarray(x), np.asarray(Wq), np.asarray(Wk),
                     np.asarray(Wv))
    res = run_bass_kernel_spmd(nc, in_maps, core_ids=list(range(8)),
                               trace=trace)
    return _unshard(res.results, np.float32), res


def kernel(x, Wq, Wk, Wv):
    out, _ = run(x, Wq, Wk, Wv, trace=False)
    return out


# revision 10
# speedup vs baseline: 1.0085x; 1.0085x over previous
"""Causal self-attention (B=4, S=2048, D=1024, single head) on 8 TRN2 cores.

Sharding: core c handles batch b = c//2 with query-tile parity p = c%2 —
its 8 query tiles of 128 rows are the absolute 128-row tiles {2j+p}.
Interleaving parities balances causal work exactly; the single SPMD
program is uniform and per-core variation is data-only (query rows and
the additive causal mask for the last kv group).

Re-associated algebra removes the k and v projections entirely:
  scores = q @ k.T = ((x_q Wq^T) Wk) @ x^T      (only q-rows projected)
  out    = attn @ v = (attn @ x) @ Wv^T          (project the context)

All matmul operands are bf16 (fp32 PSUM accumulation). Every DRAM input
is pre-arranged host-side into its exact SBUF flat layout, so each DMA
is a contiguous [128, cols] block copy ordered to match consumption.
The PE is pre-warmed with dummy matmuls during the initial DMA wait
(post-idle it runs at reduced p-state for ~3us).

PE program order (software-pipelined):
  A: qT[o,sq] = wqT.T @ xqT          B: qkT[d,sq] = wk.T @ qT
  S(0) Ptr(0) ctx(0) | S(1) | for j>=1: ctxT(j-1) Ptr(j) out(j-1)
  ctx(j) S(j+1) | ctxT(7) out(7)
where S = scores+exp (fused row-sum), Ptr = PE-transpose of P,
ctx = PT.T @ xnat (then *1/rowsum, cast bf16), ctxT = PE-transpose,
out = ctxT.T @ wvT.
"""

import numpy as np
import ml_dtypes

B, S, D = 4, 2048, 1024
DC = D // 128          # 128-chunks along d / o
NB = S // 128          # kv blocks per batch
NT = 8                 # q tiles per core
SCALE = 1.0 / np.sqrt(np.float32(D))
NEG = np.float32(-1e30)
BF16 = ml_dtypes.bfloat16

_cache = {}


def _ext(j):
    # kv extent for local tile j in 128-blocks (uniform across cores);
    # rounded up to a multiple of 2 so the tail group is 256-wide
    return 2 * j + 2


def _build():
    if "nc" in _cache:
        return _cache["nc"]

    import concourse.bacc as bacc
    import concourse.mybir as mybir
    import concourse.tile as tile

    f32 = mybir.dt.float32
    bf16 = mybir.dt.bfloat16
    AF = mybir.ActivationFunctionType

    nc = bacc.Bacc("TRN2", target_bir_lowering=False, debug=False,
                   num_devices=8)
    # all inputs pre-arranged host-side to the SBUF layout, [128, cols]
    xq_d = nc.dram_tensor("xq", [128, 2 * DC * 512], bf16,
                          kind="ExternalInput").ap()    # [p, sg,dc,s]
    xkv_d = nc.dram_tensor("xkv", [128, DC * S], bf16,
                           kind="ExternalInput").ap()   # [p, dc,kv]
    xn_d = nc.dram_tensor("xn", [128, NB * D], bf16,
                          kind="ExternalInput").ap()    # [p, kb,d]
    wq_d = nc.dram_tensor("wq", [128, DC * D], bf16,
                          kind="ExternalInput").ap()    # [p, ot,dc,oo]
    wk_d = nc.dram_tensor("wk", [128, DC * D], bf16,
                          kind="ExternalInput").ap()    # [p, dc,oc,dd]
    wv_d = nc.dram_tensor("wv", [128, DC * D], bf16,
                          kind="ExternalInput").ap()    # [p, dc,o]
    masks_d = nc.dram_tensor("masks", [128, NT * 512], bf16,
                             kind="ExternalInput").ap()  # [p, j,k]
    ident_d = nc.dram_tensor("ident", [128, 128], bf16,
                             kind="ExternalInput").ap()
    out_d = nc.dram_tensor("out", [NT * 128, D], f32,
                           kind="ExternalOutput").ap()

    with tile.TileContext(nc) as tc:
        with (
            tc.tile_pool(name="persist", bufs=1) as persist,
            tc.tile_pool(name="wpool", bufs=2) as wp,
            tc.tile_pool(name="pD", bufs=2) as pp,
            tc.tile_pool(name="ptD", bufs=1) as ptp,
            tc.tile_pool(name="cD", bufs=2) as cp,
            tc.tile_pool(name="ctD", bufs=2) as ctp,
            tc.tile_pool(name="oD", bufs=2) as op,
            tc.tile_pool(name="smD", bufs=2) as smp,
        ):
            xkvT = persist.tile([128, DC * S], bf16)
            xnat = persist.tile([128, NB * D], bf16)
            qT = persist.tile([128, DC * NT * 128], bf16)
            qkT = persist.tile([128, DC * NT * 128], bf16)
            masks = persist.tile([128, NT * 512], bf16)
            ident = persist.tile([128, 128], bf16)
            warm = persist.tile([128, 1], f32)
            scr = persist.tile([128, 512], bf16)

            def load(dst, src, cuts, eng):
                for a, b in zip(cuts[:-1], cuts[1:]):
                    eng.dma_start(dst[:, a:b], src[:, a:b])

            Ph, rch, cth = {}, {}, {}

            def scores_exp(j, pool):
                ext = _ext(j)
                ng = (ext + 3) // 4
                P = pp.tile([128, NB * 128], bf16, tag="P", name=f"P{j}")
                dslots = smp.tile([128, 4], f32, tag="ds", name=f"ds{j}")
                for g in range(ng):
                    gw = min(512, ext * 128 - g * 512)
                    last = (g == ng - 1)
                    sps = pool.tile([128, 512], f32, tag="sc",
                                    name=f"sps{j}_{g}")
                    for dc in range(DC):
                        nc.tensor.matmul(
                            sps[:, 0:gw],
                            qkT[:, dc * 1024 + j * 128:dc * 1024 + j * 128 + 128],
                            xkvT[:, dc * S + g * 512:dc * S + g * 512 + gw],
                            start=(dc == 0),
                            stop=(dc == DC - 1 and not last))
                    if last:
                        nc.tensor.matmul(
                            sps[:, 0:gw], ident[:],
                            masks[:, j * 512:j * 512 + gw],
                            start=False, stop=True)
                    nc.scalar.activation(
                        P[:, g * 512:g * 512 + gw], sps[:, 0:gw], AF.Exp,
                        scale=float(SCALE),
                        accum_out=dslots[:, g:g + 1])
                rcp = smp.tile([128, 1], f32, tag="rcp", name=f"rcp{j}")
                den = smp.tile([128, 1], f32, tag="den", name=f"den{j}")
                nc.vector.reduce_sum(den[:], dslots[:, 0:ng],
                                     axis=mybir.AxisListType.X)
                nc.vector.reciprocal(rcp[:], den[:])
                Ph[j], rch[j] = P, rcp

            with (
                tc.tile_pool(name="xqpool", bufs=1) as xqs,
                tc.tile_pool(name="psAB", bufs=4, space="PSUM") as psA,
            ):
                nc.gpsimd.memset(warm[:], 0.0)
                nc.gpsimd.memset(scr[:], 0.0)

                # ---- input DMA: contiguous pieces, first-needed first ----
                xq_sb = xqs.tile([128, 2 * DC * 512], bf16, tag="xq")
                wq = wp.tile([128, DC * D], bf16, name="w_wq", tag="w")
                wk = wp.tile([128, DC * D], bf16, name="w_wk", tag="w")
                K = 1024
                load(wq[:], wq_d, tuple(range(0, 9 * K, K)), nc.sync)
                load(xq_sb[:], xq_d, (0, 2 * K, 4 * K, 6 * K, 8 * K),
                     nc.scalar)
                nc.scalar.dma_start(ident[:], ident_d)
                # warm-up Exp table load (after the critical descriptors)
                nc.scalar.activation(warm[:], warm[:], AF.Exp)
                load(wk[:], wk_d, (0, 4 * K, 8 * K), nc.sync)
                nc.scalar.dma_start(masks[:], masks_d)
                load(xkvT[:], xkv_d, (0, 4 * K, 8 * K, 12 * K, 16 * K),
                     nc.sync)
                wv = wp.tile([128, DC * D], bf16, name="w_wv", tag="w")
                load(wv[:], wv_d, (0, 4 * K, 8 * K), nc.sync)
                load(xnat[:], xn_d, (0, 4 * K, 8 * K, 12 * K, 16 * K),
                     nc.scalar)

                # PE p-state warm-up: dummy matmuls on memset scratch,
                # sized to bridge until the first wq/xq pieces land
                for _ in range(10):
                    ps = psA.tile([128, 512], f32, tag="pj")
                    nc.tensor.matmul(ps[:], scr[:, 0:128], scr[:],
                                     start=True, stop=True)

                # ---- Phase A: q projection (into resident qT) ----
                for sg in range(2):
                    for ot in range(8):
                        ps = psA.tile([128, 512], f32, tag="pj")
                        for dc in range(DC):
                            nc.tensor.matmul(
                                ps[:],
                                wq[:, ot * 1024 + dc * 128:ot * 1024 + dc * 128 + 128],
                                xq_sb[:, sg * 4096 + dc * 512:sg * 4096 + dc * 512 + 512],
                                start=(dc == 0), stop=(dc == DC - 1))
                        nc.vector.tensor_copy(
                            qT[:, ot * 1024 + sg * 512:ot * 1024 + sg * 512 + 512],
                            ps[:])

                # ---- Phase B: fold Wk into q (qk = q @ Wk, transposed) ----
                # scores(0)/(1) are slotted into the B window so exp(0/1)
                # latency hides under B's matmuls
                for sg in range(2):
                    for dc in range(DC):
                        ps = psA.tile([128, 512], f32, tag="pj")
                        for oc in range(DC):
                            nc.tensor.matmul(
                                ps[:],
                                wk[:, dc * 1024 + oc * 128:dc * 1024 + oc * 128 + 128],
                                qT[:, oc * 1024 + sg * 512:oc * 1024 + sg * 512 + 512],
                                start=(oc == 0), stop=(oc == DC - 1))
                        nc.scalar.copy(
                            qkT[:, dc * 1024 + sg * 512:dc * 1024 + sg * 512 + 512],
                            ps[:])
                    scores_exp(sg, psA)

            # ---- attention ----
            with (
                tc.tile_pool(name="psS", bufs=2, space="PSUM") as ps_s,
                tc.tile_pool(name="psT", bufs=2, space="PSUM") as ps_t,
                tc.tile_pool(name="psM", bufs=4, space="PSUM") as ps_m,
            ):
                def ptr(j):
                    ext = _ext(j)
                    P = Ph[j]
                    PT = ptp.tile([128, NB * 128], bf16, tag="PT",
                                  name=f"PT{j}")
                    for g in range((ext + 3) // 4):
                        nb = min(4, ext - g * 4)
                        tps = ps_t.tile([128, 512], bf16, tag="tp",
                                        name=f"tps{j}_{g}")
                        for bb in range(nb):
                            nc.tensor.transpose(
                                tps[:, bb * 128:(bb + 1) * 128],
                                P[:, g * 512 + bb * 128:g * 512 + bb * 128 + 128],
                                ident[:])
                        nc.vector.tensor_copy(
                            PT[:, g * 512:g * 512 + nb * 128],
                            tps[:, 0:nb * 128])
                    return PT

                def ctx_mm(j, PT):
                    ext = _ext(j)
                    ctx = cp.tile([128, D], bf16, tag="ctx", name=f"ctx{j}")
                    for og in range(2):
                        ops = ps_m.tile([128, 512], f32, tag="av",
                                        name=f"av{j}_{og}")
                        for kb in range(ext):
                            nc.tensor.matmul(
                                ops[:],
                                PT[:, kb * 128:(kb + 1) * 128],
                                xnat[:, kb * D + og * 512:kb * D + og * 512 + 512],
                                start=(kb == 0), stop=(kb == ext - 1))
                        # normalize+cast on ScalarE to keep VectorE free for
                        # the PT/ctxT copies that gate the PE
                        nc.scalar.mul(
                            ctx[:, og * 512:(og + 1) * 512], ops[:],
                            rch[j][:])
                    cth[j] = ctx

                def ctxT_tr(j):
                    ctx = cth[j]
                    ctxT = ctp.tile([128, D], bf16, tag="ctxT",
                                    name=f"ctxT{j}")
                    for h in range(2):
                        tps = ps_t.tile([128, 512], bf16, tag="tp",
                                        name=f"tpc{j}_{h}")
                        for q4 in range(4):
                            dc = h * 4 + q4
                            nc.tensor.transpose(
                                tps[:, q4 * 128:(q4 + 1) * 128],
                                ctx[:, dc * 128:dc * 128 + 128],
                                ident[:])
                        nc.vector.tensor_copy(
                            ctxT[:, h * 512:(h + 1) * 512], tps[:])
                    return ctxT

                def out_mm(j, ctxT):
                    osb = op.tile([128, D], f32, tag="o", name=f"o{j}")
                    out3 = out_d.rearrange("q (og o) -> q og o", og=2)
                    for og in range(2):
                        ops = ps_m.tile([128, 512], f32, tag="av",
                                        name=f"op{j}_{og}")
                        for dc in range(DC):
                            nc.tensor.matmul(
                                ops[:],
                                ctxT[:, dc * 128:dc * 128 + 128],
                                wv[:, dc * 1024 + og * 512:dc * 1024 + og * 512 + 512],
                                start=(dc == 0), stop=(dc == DC - 1))
                        nc.vector.tensor_copy(
                            osb[:, og * 512:(og + 1) * 512], ops[:])
                        nc.sync.dma_start(
                            out3[j * 128:(j + 1) * 128, og],
                            osb[:, og * 512:(og + 1) * 512])

                # software pipeline (S(0)/S(1) already issued in B window):
                #   Ptr(0) ctx(0) | j: ctxT(j-1) Ptr(j) out(j-1) ctx(j)
                #   S(j+1) | ctxT(7) out(7)
                PT = ptr(0)
                ctx_mm(0, PT)
                for j in range(1, NT):
                    cT = ctxT_tr(j - 1)
                    PT = ptr(j)
                    out_mm(j - 1, cT)
                    ctx_mm(j, PT)
                    if j + 1 < NT:
                        scores_exp(j + 1, ps_s)
                cT = ctxT_tr(NT - 1)
                out_mm(NT - 1, cT)

    nc.compile()
    _cache["nc"] = nc
    return nc


def _shard(x, Wq, Wk, Wv):
    """Build the 8 per-core input maps, pre-arranged to SBUF layouts."""
    ident = np.eye(128, dtype=np.float32).astype(BF16)
    # wq2[p, ot*1024+dc*128+oo] = Wq[ot*128+oo, dc*128+p]
    wq2 = np.ascontiguousarray(
        Wq.reshape(8, 128, 8, 128).transpose(3, 0, 2, 1)
        .reshape(128, 8192)).astype(BF16)
    # wk2[p, dc*1024+oc*128+dd] = Wk[oc*128+p, dc*128+dd]
    wk2 = np.ascontiguousarray(
        Wk.reshape(8, 128, 8, 128).transpose(1, 2, 0, 3)
        .reshape(128, 8192)).astype(BF16)
    # wv2[p, dc*1024+o] = Wv[o, dc*128+p]
    wv2 = np.ascontiguousarray(
        Wv.T.reshape(8, 128, 1024).transpose(1, 0, 2)
        .reshape(128, 8192)).astype(BF16)
    in_maps = []
    for c in range(8):
        b, p = c // 2, c % 2
        xb = np.asarray(x[b])
        # xkv2[p, dc*2048+kv] = x[kv, dc*128+p]
        xkv2 = np.ascontiguousarray(
            xb.T.reshape(8, 128, 2048).transpose(1, 0, 2)
            .reshape(128, 16384)).astype(BF16)
        # xn2[p, kb*1024+d] = x[kb*128+p, d]
        xn2 = np.ascontiguousarray(
            xb.reshape(16, 128, 1024).transpose(1, 0, 2)
            .reshape(128, 16384)).astype(BF16)
        rows = np.concatenate(
            [xb[(2 * j + p) * 128:(2 * j + p + 1) * 128, :] for j in range(8)],
            axis=0)
        # xq2[p, sg*4096+dc*512+ss] = rows[sg*512+ss, dc*128+p]
        xq2 = np.ascontiguousarray(
            rows.reshape(2, 512, 8, 128).transpose(3, 0, 2, 1)
            .reshape(128, 8192)).astype(BF16)
        masks = np.full((NT * 128, 512), NEG, np.float32)
        for j in range(NT):
            ext = _ext(j)
            ng = (ext + 3) // 4
            gw = min(512, ext * 128 - (ng - 1) * 512)
            q_abs = (2 * j + p) * 128 + np.arange(128)[:, None]
            kv_abs = (ng - 1) * 512 + np.arange(gw)[None, :]
            masks[j * 128:(j + 1) * 128, 0:gw] = np.where(
                kv_abs <= q_abs, np.float32(0), NEG)
        masks2 = np.ascontiguousarray(
            masks.reshape(8, 128, 512).transpose(1, 0, 2)
            .reshape(128, 4096)).astype(BF16)
        in_maps.append({
            "xq": xq2, "xkv": xkv2, "xn": xn2,
            "wq": wq2, "wk": wk2, "wv": wv2,
            "masks": masks2, "ident": ident,
        })
    return in_maps


def _unshard(results, dtype):
    out = np.empty((B, S, D), dtype)
    for c in range(8):
        b, p = c // 2, c % 2
        o = results[c]["out"]
        for j in range(NT):
            out[b, (2 * j + p) * 128:(2 * j + p + 1) * 128, :] = \
                o[j * 128:(j + 1) * 128, :]
    return out


def run(x, Wq, Wk, Wv, trace=False):
    from concourse.bass_utils import run_bass_kernel_spmd
    nc = _build()
    in_maps = _shard(np.asarray(x), np.asarray(Wq), np.asarray(Wk),
                     np.asarray(Wv))
    res = run_bass_kernel_spmd(nc, in_maps, core_ids=list(range(8)),
                               trace=trace)
    return _unshard(res.results, np.float32), res


def kernel(x, Wq, Wk, Wv):
    out, _ = run(x, Wq, Wk, Wv, trace=False)
    return out


# revision 13
# speedup vs baseline: 1.0134x; 1.0049x over previous
"""Causal self-attention (B=4, S=2048, D=1024, single head) on 8 TRN2 cores.

Sharding: core c handles batch b = c//2 with query-tile parity p = c%2 —
its 8 query tiles of 128 rows are the absolute 128-row tiles {2j+p}.
Interleaving parities balances causal work exactly; the single SPMD
program is uniform and per-core variation is data-only (query rows and
the additive causal mask for the last kv group).

Re-associated algebra removes the k and v projections entirely:
  scores = q @ k.T = ((x_q Wq^T) Wk) @ x^T      (only q-rows projected)
  out    = attn @ v = (attn @ x) @ Wv^T          (project the context)

All matmul operands are bf16 (fp32 PSUM accumulation). Every DRAM input
is pre-arranged host-side into its exact SBUF flat layout, so each DMA
is a contiguous [128, cols] block copy ordered to match consumption.
The PE is pre-warmed with dummy matmuls during the initial DMA wait
(post-idle it runs at reduced p-state for ~3us).

PE program order (software-pipelined):
  A: qT[o,sq] = wqT.T @ xqT          B: qkT[d,sq] = wk.T @ qT
  S(0) Ptr(0) ctx(0) | S(1) | for j>=1: ctxT(j-1) Ptr(j) out(j-1)
  ctx(j) S(j+1) | ctxT(7) out(7)
where S = scores+exp (fused row-sum), Ptr = PE-transpose of P,
ctx = PT.T @ xnat (then *1/rowsum, cast bf16), ctxT = PE-transpose,
out = ctxT.T @ wvT.
"""

import numpy as np
import ml_dtypes

B, S, D = 4, 2048, 1024
DC = D // 128          # 128-chunks along d / o
NB = S // 128          # kv blocks per batch
NT = 8                 # q tiles per core
SCALE = 1.0 / np.sqrt(np.float32(D))
NEG = np.float32(-1e30)
BF16 = ml_dtypes.bfloat16

_cache = {}


def _ext(j):
    # kv extent for local tile j in 128-blocks (uniform across cores);
    # rounded up to a multiple of 2 so the tail group is 256-wide
    return 2 * j + 2


def _build():
    if "nc" in _cache:
        return _cache["nc"]

    import concourse.bacc as bacc
    import concourse.mybir as mybir
    import concourse.tile as tile

    f32 = mybir.dt.float32
    bf16 = mybir.dt.bfloat16
    AF = mybir.ActivationFunctionType

    nc = bacc.Bacc("TRN2", target_bir_lowering=False, debug=False,
                   num_devices=8)
    # all inputs pre-arranged host-side to the SBUF layout, [128, cols]
    xq_d = nc.dram_tensor("xq", [128, 2 * DC * 512], bf16,
                          kind="ExternalInput").ap()    # [p, sg,dc,s]
    xkv_d = nc.dram_tensor("xkv", [128, DC * S], bf16,
                           kind="ExternalInput").ap()   # [p, dc,kv]
    xn_d = nc.dram_tensor("xn", [128, NB * D], bf16,
                          kind="ExternalInput").ap()    # [p, kb,d]
    wq_d = nc.dram_tensor("wq", [128, DC * D], bf16,
                          kind="ExternalInput").ap()    # [p, ot,dc,oo]
    wk_d = nc.dram_tensor("wk", [128, DC * D], bf16,
                          kind="ExternalInput").ap()    # [p, dc,oc,dd]
    wv_d = nc.dram_tensor("wv", [128, DC * D], bf16,
                          kind="ExternalInput").ap()    # [p, dc,o]
    masks_d = nc.dram_tensor("masks", [128, NT * 512], bf16,
                             kind="ExternalInput").ap()  # [p, j,k]
    ident_d = nc.dram_tensor("ident", [128, 128], bf16,
                             kind="ExternalInput").ap()
    out_d = nc.dram_tensor("out", [NT * 128, D], f32,
                           kind="ExternalOutput").ap()

    with tile.TileContext(nc) as tc:
        with (
            tc.tile_pool(name="persist", bufs=1) as persist,
            tc.tile_pool(name="wpool", bufs=2) as wp,
            tc.tile_pool(name="pD", bufs=2) as pp,
            tc.tile_pool(name="ptD", bufs=1) as ptp,
            tc.tile_pool(name="cD", bufs=2) as cp,
            tc.tile_pool(name="ctD", bufs=2) as ctp,
            tc.tile_pool(name="oD", bufs=2) as op,
            tc.tile_pool(name="smD", bufs=2) as smp,
            tc.tile_pool(name="psS", bufs=2, space="PSUM") as ps_s,
        ):
            xkvT = persist.tile([128, DC * S], bf16)
            xnat = persist.tile([128, NB * D], bf16)
            qT = persist.tile([128, DC * NT * 128], bf16)
            qkT = persist.tile([128, DC * NT * 128], bf16)
            masks = persist.tile([128, NT * 512], bf16)
            ident = persist.tile([128, 128], bf16)
            warm = persist.tile([128, 1], f32)
            scr = persist.tile([128, 512], bf16)

            def load(dst, src, cuts, eng):
                for a, b in zip(cuts[:-1], cuts[1:]):
                    eng.dma_start(dst[:, a:b], src[:, a:b])

            Ph, rch, cth = {}, {}, {}

            def scores_exp(j, pool):
                ext = _ext(j)
                ng = (ext + 3) // 4
                P = pp.tile([128, NB * 128], bf16, tag="P", name=f"P{j}")
                dslots = smp.tile([128, 4], f32, tag="ds", name=f"ds{j}")
                for g in range(ng):
                    gw = min(512, ext * 128 - g * 512)
                    last = (g == ng - 1)
                    sps = pool.tile([128, 512], f32, tag="sc",
                                    name=f"sps{j}_{g}")
                    for dc in range(DC):
                        nc.tensor.matmul(
                            sps[:, 0:gw],
                            qkT[:, dc * 1024 + j * 128:dc * 1024 + j * 128 + 128],
                            xkvT[:, dc * S + g * 512:dc * S + g * 512 + gw],
                            start=(dc == 0),
                            stop=(dc == DC - 1 and not last))
                    if last:
                        nc.tensor.matmul(
                            sps[:, 0:gw], ident[:],
                            masks[:, j * 512:j * 512 + gw],
                            start=False, stop=True)
                    nc.scalar.activation(
                        P[:, g * 512:g * 512 + gw], sps[:, 0:gw], AF.Exp,
                        scale=float(SCALE),
                        accum_out=dslots[:, g:g + 1])
                rcp = smp.tile([128, 1], f32, tag="rcp", name=f"rcp{j}")
                den = smp.tile([128, 1], f32, tag="den", name=f"den{j}")
                nc.vector.reduce_sum(den[:], dslots[:, 0:ng],
                                     axis=mybir.AxisListType.X)
                nc.vector.reciprocal(rcp[:], den[:])
                Ph[j], rch[j] = P, rcp

            with (
                tc.tile_pool(name="xqpool", bufs=1) as xqs,
                tc.tile_pool(name="psAB", bufs=4, space="PSUM") as psA,
            ):
                nc.gpsimd.memset(warm[:], 0.0)
                nc.gpsimd.memset(scr[:], 0.0)

                # ---- input DMA: contiguous pieces, first-needed first ----
                xq_sb = xqs.tile([128, 2 * DC * 512], bf16, tag="xq")
                wq = wp.tile([128, DC * D], bf16, name="w_wq", tag="w")
                wk = wp.tile([128, DC * D], bf16, name="w_wk", tag="w")
                K = 1024
                # critical path: A(ot0) needs xq cols 0:4096 (sg0) + wq ot0;
                # split sg0 across both queues to beat per-queue DGE BW
                nc.scalar.dma_start(xq_sb[:, 0:2 * K], xq_d[:, 0:2 * K])
                nc.sync.dma_start(xq_sb[:, 2 * K:4 * K], xq_d[:, 2 * K:4 * K])
                for i in range(8):
                    eng = nc.sync if i % 2 == 0 else nc.scalar
                    eng.dma_start(wq[:, i * K:(i + 1) * K],
                                  wq_d[:, i * K:(i + 1) * K])
                nc.scalar.dma_start(xq_sb[:, 4 * K:6 * K], xq_d[:, 4 * K:6 * K])
                nc.sync.dma_start(xq_sb[:, 6 * K:8 * K], xq_d[:, 6 * K:8 * K])
                nc.scalar.dma_start(ident[:], ident_d)
                # warm-up Exp table load (after the critical descriptors)
                nc.scalar.activation(warm[:], warm[:], AF.Exp)
                load(wk[:], wk_d, (0, 4 * K, 8 * K), nc.sync)
                nc.scalar.dma_start(masks[:], masks_d)
                load(xkvT[:], xkv_d, (0, 4 * K, 8 * K, 12 * K, 16 * K),
                     nc.sync)
                wv = wp.tile([128, DC * D], bf16, name="w_wv", tag="w")
                load(wv[:], wv_d, (0, 4 * K, 8 * K), nc.sync)
                load(xnat[:], xn_d, (0, 4 * K, 8 * K, 12 * K, 16 * K),
                     nc.scalar)

                # PE p-state warm-up: dummy matmuls on memset scratch,
                # sized to bridge until the first wq/xq pieces land
                for _ in range(10):
                    ps = psA.tile([128, 512], f32, tag="pj")
                    nc.tensor.matmul(ps[:], scr[:, 0:128], scr[:],
                                     start=True, stop=True)

                # ---- Phase A: q projection (into resident qT) ----
                for sg in range(2):
                    for ot in range(8):
                        ps = psA.tile([128, 512], f32, tag="pj")
                        for dc in range(DC):
                            nc.tensor.matmul(
                                ps[:],
                                wq[:, ot * 1024 + dc * 128:ot * 1024 + dc * 128 + 128],
                                xq_sb[:, sg * 4096 + dc * 512:sg * 4096 + dc * 512 + 512],
                                start=(dc == 0), stop=(dc == DC - 1))
                        nc.vector.tensor_copy(
                            qT[:, ot * 1024 + sg * 512:ot * 1024 + sg * 512 + 512],
                            ps[:])

                # ---- Phase B: fold Wk into q (qk = q @ Wk, transposed) ----
                # scores(0)/(1) are slotted into the B window so exp(0/1)
                # latency hides under B's matmuls
                for sg in range(2):
                    for dc in range(DC):
                        ps = psA.tile([128, 512], f32, tag="pj")
                        for oc in range(DC):
                            nc.tensor.matmul(
                                ps[:],
                                wk[:, dc * 1024 + oc * 128:dc * 1024 + oc * 128 + 128],
                                qT[:, oc * 1024 + sg * 512:oc * 1024 + sg * 512 + 512],
                                start=(oc == 0), stop=(oc == DC - 1))
                        nc.scalar.copy(
                            qkT[:, dc * 1024 + sg * 512:dc * 1024 + sg * 512 + 512],
                            ps[:])
                    scores_exp(sg, ps_s)

            # ---- attention ----
            with (
                tc.tile_pool(name="psT", bufs=2, space="PSUM") as ps_t,
                tc.tile_pool(name="psM", bufs=4, space="PSUM") as ps_m,
            ):
                def ptr(j):
                    ext = _ext(j)
                    P = Ph[j]
                    PT = ptp.tile([128, NB * 128], bf16, tag="PT",
                                  name=f"PT{j}")
                    for g in range((ext + 3) // 4):
                        nb = min(4, ext - g * 4)
                        tps = ps_t.tile([128, 512], bf16, tag="tp",
                                        name=f"tps{j}_{g}")
                        for bb in range(nb):
                            nc.tensor.transpose(
                                tps[:, bb * 128:(bb + 1) * 128],
                                P[:, g * 512 + bb * 128:g * 512 + bb * 128 + 128],
                                ident[:])
                        nc.vector.tensor_copy(
                            PT[:, g * 512:g * 512 + nb * 128],
                            tps[:, 0:nb * 128])
                    return PT

                def ctx_mm(j, PT):
                    ext = _ext(j)
                    ctx = cp.tile([128, D], bf16, tag="ctx", name=f"ctx{j}")
                    for og in range(2):
                        ops = ps_m.tile([128, 512], f32, tag="av",
                                        name=f"av{j}_{og}")
                        for kb in range(ext):
                            nc.tensor.matmul(
                                ops[:],
                                PT[:, kb * 128:(kb + 1) * 128],
                                xnat[:, kb * D + og * 512:kb * D + og * 512 + 512],
                                start=(kb == 0), stop=(kb == ext - 1))
                        # normalize+cast on ScalarE to keep VectorE free for
                        # the PT/ctxT copies that gate the PE
                        nc.scalar.mul(
                            ctx[:, og * 512:(og + 1) * 512], ops[:],
                            rch[j][:])
                    cth[j] = ctx

                def ctxT_tr(j):
                    ctx = cth[j]
                    ctxT = ctp.tile([128, D], bf16, tag="ctxT",
                                    name=f"ctxT{j}")
                    for h in range(2):
                        tps = ps_t.tile([128, 512], bf16, tag="tp",
                                        name=f"tpc{j}_{h}")
                        for q4 in range(4):
                            dc = h * 4 + q4
                            nc.tensor.transpose(
                                tps[:, q4 * 128:(q4 + 1) * 128],
                                ctx[:, dc * 128:dc * 128 + 128],
                                ident[:])
                        nc.vector.tensor_copy(
                            ctxT[:, h * 512:(h + 1) * 512], tps[:])
                    return ctxT

                def out_mm(j, ctxT):
                    osb = op.tile([128, D], f32, tag="o", name=f"o{j}")
                    out3 = out_d.rearrange("q (og o) -> q og o", og=2)
                    for og in range(2):
                        ops = ps_m.tile([128, 512], f32, tag="av",
                                        name=f"op{j}_{og}")
                        for dc in range(DC):
                            nc.tensor.matmul(
                                ops[:],
                                ctxT[:, dc * 128:dc * 128 + 128],
                                wv[:, dc * 1024 + og * 512:dc * 1024 + og * 512 + 512],
                                start=(dc == 0), stop=(dc == DC - 1))
                        nc.vector.tensor_copy(
                            osb[:, og * 512:(og + 1) * 512], ops[:])
                        nc.sync.dma_start(
                            out3[j * 128:(j + 1) * 128, og],
                            osb[:, og * 512:(og + 1) * 512])

                # software pipeline (S(0)/S(1) already issued in B window):
                #   Ptr(0) ctx(0) | j: ctxT(j-1) Ptr(j) out(j-1) ctx(j)
                #   S(j+1) | ctxT(7) out(7)
                PT = ptr(0)
                ctx_mm(0, PT)
                for j in range(1, NT):
                    cT = ctxT_tr(j - 1)
                    PT = ptr(j)
                    out_mm(j - 1, cT)
                    ctx_mm(j, PT)
                    if j + 1 < NT:
                        scores_exp(j + 1, ps_s)
                cT = ctxT_tr(NT - 1)
                out_mm(NT - 1, cT)

    nc.compile()
    _cache["nc"] = nc
    return nc


def _shard(x, Wq, Wk, Wv):
    """Build the 8 per-core input maps, pre-arranged to SBUF layouts."""
    ident = np.eye(128, dtype=np.float32).astype(BF16)
    # wq2[p, ot*1024+dc*128+oo] = Wq[ot*128+oo, dc*128+p]
    wq2 = np.ascontiguousarray(
        Wq.reshape(8, 128, 8, 128).transpose(3, 0, 2, 1)
        .reshape(128, 8192)).astype(BF16)
    # wk2[p, dc*1024+oc*128+dd] = Wk[oc*128+p, dc*128+dd]
    wk2 = np.ascontiguousarray(
        Wk.reshape(8, 128, 8, 128).transpose(1, 2, 0, 3)
        .reshape(128, 8192)).astype(BF16)
    # wv2[p, dc*1024+o] = Wv[o, dc*128+p]
    wv2 = np.ascontiguousarray(
        Wv.T.reshape(8, 128, 1024).transpose(1, 0, 2)
        .reshape(128, 8192)).astype(BF16)
    in_maps = []
    for c in range(8):
        b, p = c // 2, c % 2
        xb = np.asarray(x[b])
        # xkv2[p, dc*2048+kv] = x[kv, dc*128+p]
        xkv2 = np.ascontiguousarray(
            xb.T.reshape(8, 128, 2048).transpose(1, 0, 2)
            .reshape(128, 16384)).astype(BF16)
        # xn2[p, kb*1024+d] = x[kb*128+p, d]
        xn2 = np.ascontiguousarray(
            xb.reshape(16, 128, 1024).transpose(1, 0, 2)
            .reshape(128, 16384)).astype(BF16)
        rows = np.concatenate(
            [xb[(2 * j + p) * 128:(2 * j + p + 1) * 128, :] for j in range(8)],
            axis=0)
        # xq2[p, sg*4096+dc*512+ss] = rows[sg*512+ss, dc*128+p]
        xq2 = np.ascontiguousarray(
            rows.reshape(2, 512, 8, 128).transpose(3, 0, 2, 1)
            .reshape(128, 8192)).astype(BF16)
        masks = np.full((NT * 128, 512), NEG, np.float32)
        for j in range(NT):
            ext = _ext(j)
            ng = (ext + 3) // 4
            gw = min(512, ext * 128 - (ng - 1) * 512)
            q_abs = (2 * j + p) * 128 + np.arange(128)[:, None]
            kv_abs = (ng - 1) * 512 + np.arange(gw)[None, :]
            masks[j * 128:(j + 1) * 128, 0:gw] = np.where(
                kv_abs <= q_abs, np.float32(0), NEG)
        masks2 = np.ascontiguousarray(
            masks.reshape(8, 128, 512).transpose(1, 0, 2)
            .reshape(128, 4096)).astype(BF16)
        in_maps.append({
            "xq": xq2, "xkv": xkv2, "xn": xn2,
            "wq": wq2, "wk": wk2, "wv": wv2,
            "masks": masks2, "ident": ident,
        })
    return in_maps


def _unshard(results, dtype):
    out = np.empty((B, S, D), dtype)
    for c in range(8):
        b, p = c // 2, c % 2
        o = results[c]["out"]
        for j in range(NT):
            out[b, (2 * j + p) * 128:(2 * j + p + 1) * 128, :] = \
                o[j * 128:(j + 1) * 128, :]
    return out


def run(x, Wq, Wk, Wv, trace=False):
    from concourse.bass_utils import run_bass_kernel_spmd
    nc = _build()
    in_maps = _shard(np.asarray(x), np.asarray(Wq), np.asarray(Wk),
                     np.asarray(Wv))
    res = run_bass_kernel_spmd(nc, in_maps, core_ids=list(range(8)),
                               trace=trace)
    return _unshard(res.results, np.float32), res


def kernel(x, Wq, Wk, Wv):
    out, _ = run(x, Wq, Wk, Wv, trace=False)
    return out


# revision 15
# speedup vs baseline: 1.0270x; 1.0134x over previous
"""Causal self-attention (B=4, S=2048, D=1024, single head) on 8 TRN2 cores.

Sharding: core c handles batch b = c//2 with query-tile parity p = c%2 —
its 8 query tiles of 128 rows are the absolute 128-row tiles {2j+p}.
Interleaving parities balances causal work exactly; the single SPMD
program is uniform and per-core variation is data-only (query rows and
the additive causal mask for the last kv group).

Re-associated algebra removes the k and v projections entirely:
  scores = q @ k.T = ((x_q Wq^T) Wk) @ x^T      (only q-rows projected)
  out    = attn @ v = (attn @ x) @ Wv^T          (project the context)

All matmul operands are bf16 (fp32 PSUM accumulation). Every DRAM input
is pre-arranged host-side into its exact SBUF flat layout, so each DMA
is a contiguous [128, cols] block copy ordered to match consumption.
The PE is pre-warmed with dummy matmuls during the initial DMA wait
(post-idle it runs at reduced p-state for ~3us).

PE program order (software-pipelined):
  A: qT[o,sq] = wqT.T @ xqT          B: qkT[d,sq] = wk.T @ qT
  S(0) Ptr(0) ctx(0) | S(1) | for j>=1: ctxT(j-1) Ptr(j) out(j-1)
  ctx(j) S(j+1) | ctxT(7) out(7)
where S = scores+exp (fused row-sum), Ptr = PE-transpose of P,
ctx = PT.T @ xnat (then *1/rowsum, cast bf16), ctxT = PE-transpose,
out = ctxT.T @ wvT.
"""

import numpy as np
import ml_dtypes

B, S, D = 4, 2048, 1024
DC = D // 128          # 128-chunks along d / o
NB = S // 128          # kv blocks per batch
NT = 8                 # q tiles per core
SCALE = 1.0 / np.sqrt(np.float32(D))
NEG = np.float32(-1e30)
BF16 = ml_dtypes.bfloat16

_cache = {}


def _ext(j):
    # kv extent for local tile j in 128-blocks (uniform across cores);
    # rounded up to a multiple of 2 so the tail group is 256-wide
    return 2 * j + 2


def _build():
    if "nc" in _cache:
        return _cache["nc"]

    import concourse.bacc as bacc
    import concourse.mybir as mybir
    import concourse.tile as tile

    f32 = mybir.dt.float32
    bf16 = mybir.dt.bfloat16
    AF = mybir.ActivationFunctionType

    nc = bacc.Bacc("TRN2", target_bir_lowering=False, debug=False,
                   num_devices=8)
    # all inputs pre-arranged host-side to the SBUF layout, [128, cols]
    xq_d = nc.dram_tensor("xq", [128, 2 * DC * 512], bf16,
                          kind="ExternalInput").ap()    # [p, sg,dc,s]
    xkv_d = nc.dram_tensor("xkv", [128, DC * S], bf16,
                           kind="ExternalInput").ap()   # [p, dc,kv]
    xn_d = nc.dram_tensor("xn", [128, NB * D], bf16,
                          kind="ExternalInput").ap()    # [p, kb,d]
    wq_d = nc.dram_tensor("wq", [128, DC * D], bf16,
                          kind="ExternalInput").ap()    # [p, ot,dc,oo]
    wk_d = nc.dram_tensor("wk", [128, DC * D], bf16,
                          kind="ExternalInput").ap()    # [p, dc,oc,dd]
    wv_d = nc.dram_tensor("wv", [128, DC * D], bf16,
                          kind="ExternalInput").ap()    # [p, dc,o]
    masks_d = nc.dram_tensor("masks", [128, NT * 512], bf16,
                             kind="ExternalInput").ap()  # [p, j,k]
    ident_d = nc.dram_tensor("ident", [128, 128], bf16,
                             kind="ExternalInput").ap()
    out_d = nc.dram_tensor("out", [NT * 128, D], f32,
                           kind="ExternalOutput").ap()

    with tile.TileContext(nc) as tc:
        with (
            tc.tile_pool(name="persist", bufs=1) as persist,
            tc.tile_pool(name="wpool", bufs=2) as wp,
            tc.tile_pool(name="pD", bufs=2) as pp,
            tc.tile_pool(name="ptD", bufs=1) as ptp,
            tc.tile_pool(name="cD", bufs=2) as cp,
            tc.tile_pool(name="ctD", bufs=2) as ctp,
            tc.tile_pool(name="oD", bufs=2) as op,
            tc.tile_pool(name="smD", bufs=2) as smp,
            tc.tile_pool(name="psS", bufs=2, space="PSUM") as ps_s,
        ):
            xkvT = persist.tile([128, DC * S], bf16)
            xnat = persist.tile([128, NB * D], bf16)
            qT = persist.tile([128, DC * NT * 128], bf16)
            qkT = persist.tile([128, DC * NT * 128], bf16)
            masks = persist.tile([128, NT * 512], bf16)
            ident = persist.tile([128, 128], bf16)
            warm = persist.tile([128, 1], f32)
            scr = persist.tile([128, 512], bf16)

            def load(dst, src, cuts, eng):
                for a, b in zip(cuts[:-1], cuts[1:]):
                    eng.dma_start(dst[:, a:b], src[:, a:b])

            Ph, rch, cth = {}, {}, {}

            def scores_exp(j, pool):
                ext = _ext(j)
                ng = (ext + 3) // 4
                P = pp.tile([128, NB * 128], bf16, tag="P", name=f"P{j}")
                dslots = smp.tile([128, 4], f32, tag="ds", name=f"ds{j}")
                for g in range(ng):
                    gw = min(512, ext * 128 - g * 512)
                    last = (g == ng - 1)
                    sps = pool.tile([128, 512], f32, tag="sc",
                                    name=f"sps{j}_{g}")
                    for dc in range(DC):
                        nc.tensor.matmul(
                            sps[:, 0:gw],
                            qkT[:, dc * 1024 + j * 128:dc * 1024 + j * 128 + 128],
                            xkvT[:, dc * S + g * 512:dc * S + g * 512 + gw],
                            start=(dc == 0),
                            stop=(dc == DC - 1 and not last))
                    if last:
                        nc.tensor.matmul(
                            sps[:, 0:gw], ident[:],
                            masks[:, j * 512:j * 512 + gw],
                            start=False, stop=True)
                    nc.scalar.activation(
                        P[:, g * 512:g * 512 + gw], sps[:, 0:gw], AF.Exp,
                        scale=float(SCALE),
                        accum_out=dslots[:, g:g + 1])
                rcp = smp.tile([128, 1], f32, tag="rcp", name=f"rcp{j}")
                den = smp.tile([128, 1], f32, tag="den", name=f"den{j}")
                nc.vector.reduce_sum(den[:], dslots[:, 0:ng],
                                     axis=mybir.AxisListType.X)
                nc.vector.reciprocal(rcp[:], den[:])
                Ph[j], rch[j] = P, rcp

            with (
                tc.tile_pool(name="xqpool", bufs=1) as xqs,
                tc.tile_pool(name="psAB", bufs=4, space="PSUM") as psA,
            ):
                nc.gpsimd.memset(warm[:], 0.0)
                nc.gpsimd.memset(scr[:], 0.0)

                # ---- input DMA: contiguous pieces, first-needed first ----
                xq_sb = xqs.tile([128, 2 * DC * 512], bf16, tag="xq")
                wq = wp.tile([128, DC * D], bf16, name="w_wq", tag="w")
                wk = wp.tile([128, DC * D], bf16, name="w_wk", tag="w")
                K = 1024
                # critical path: A(ot0) needs xq cols 0:4096 (sg0) + wq ot0;
                # split sg0 across both queues to beat per-queue DGE BW
                nc.scalar.dma_start(xq_sb[:, 0:2 * K], xq_d[:, 0:2 * K])
                nc.sync.dma_start(xq_sb[:, 2 * K:4 * K], xq_d[:, 2 * K:4 * K])
                for i in range(8):
                    eng = nc.sync if i % 2 == 0 else nc.scalar
                    eng.dma_start(wq[:, i * K:(i + 1) * K],
                                  wq_d[:, i * K:(i + 1) * K])
                nc.scalar.dma_start(xq_sb[:, 4 * K:6 * K], xq_d[:, 4 * K:6 * K])
                nc.sync.dma_start(xq_sb[:, 6 * K:8 * K], xq_d[:, 6 * K:8 * K])
                nc.scalar.dma_start(ident[:], ident_d)
                # warm-up Exp table load (after the critical descriptors)
                nc.scalar.activation(warm[:], warm[:], AF.Exp)
                load(wk[:], wk_d, (0, 4 * K, 8 * K), nc.sync)
                nc.scalar.dma_start(masks[:], masks_d)
                load(xkvT[:], xkv_d, (0, 4 * K, 8 * K, 12 * K, 16 * K),
                     nc.sync)
                wv = wp.tile([128, DC * D], bf16, name="w_wv", tag="w")
                load(wv[:], wv_d, (0, 4 * K, 8 * K), nc.sync)
                load(xnat[:], xn_d, (0, 4 * K, 8 * K, 12 * K, 16 * K),
                     nc.scalar)

                # PE p-state warm-up: dummy matmuls on memset scratch,
                # sized to bridge until the first wq/xq pieces land
                for _ in range(20):
                    ps = psA.tile([128, 512], f32, tag="pj")
                    nc.tensor.matmul(ps[:], scr[:, 0:128], scr[:],
                                     start=True, stop=True)

                # ---- Phase A: q projection (into resident qT) ----
                for sg in range(2):
                    for ot in range(8):
                        ps = psA.tile([128, 512], f32, tag="pj")
                        for dc in range(DC):
                            nc.tensor.matmul(
                                ps[:],
                                wq[:, ot * 1024 + dc * 128:ot * 1024 + dc * 128 + 128],
                                xq_sb[:, sg * 4096 + dc * 512:sg * 4096 + dc * 512 + 512],
                                start=(dc == 0), stop=(dc == DC - 1))
                        nc.vector.tensor_copy(
                            qT[:, ot * 1024 + sg * 512:ot * 1024 + sg * 512 + 512],
                            ps[:])

                # ---- Phase B: fold Wk into q (qk = q @ Wk, transposed) ----
                # scores(0)/(1) are slotted into the B window so exp(0/1)
                # latency hides under B's matmuls
                for sg in range(2):
                    for dc in range(DC):
                        ps = psA.tile([128, 512], f32, tag="pj")
                        for oc in range(DC):
                            nc.tensor.matmul(
                                ps[:],
                                wk[:, dc * 1024 + oc * 128:dc * 1024 + oc * 128 + 128],
                                qT[:, oc * 1024 + sg * 512:oc * 1024 + sg * 512 + 512],
                                start=(oc == 0), stop=(oc == DC - 1))
                        nc.scalar.copy(
                            qkT[:, dc * 1024 + sg * 512:dc * 1024 + sg * 512 + 512],
                            ps[:])
                    scores_exp(sg, ps_s)

            # ---- attention ----
            with (
                tc.tile_pool(name="psT", bufs=2, space="PSUM") as ps_t,
                tc.tile_pool(name="psM", bufs=4, space="PSUM") as ps_m,
            ):
                def ptr(j):
                    ext = _ext(j)
                    P = Ph[j]
                    PT = ptp.tile([128, NB * 128], bf16, tag="PT",
                                  name=f"PT{j}")
                    for g in range((ext + 3) // 4):
                        nb = min(4, ext - g * 4)
                        tps = ps_t.tile([128, 512], bf16, tag="tp",
                                        name=f"tps{j}_{g}")
                        for bb in range(nb):
                            nc.tensor.transpose(
                                tps[:, bb * 128:(bb + 1) * 128],
                                P[:, g * 512 + bb * 128:g * 512 + bb * 128 + 128],
                                ident[:])
                        nc.vector.tensor_copy(
                            PT[:, g * 512:g * 512 + nb * 128],
                            tps[:, 0:nb * 128])
                    return PT

                def ctx_mm(j, PT):
                    ext = _ext(j)
                    ctx = cp.tile([128, D], bf16, tag="ctx", name=f"ctx{j}")
                    for og in range(2):
                        ops = ps_m.tile([128, 512], f32, tag="av",
                                        name=f"av{j}_{og}")
                        for kb in range(ext):
                            nc.tensor.matmul(
                                ops[:],
                                PT[:, kb * 128:(kb + 1) * 128],
                                xnat[:, kb * D + og * 512:kb * D + og * 512 + 512],
                                start=(kb == 0), stop=(kb == ext - 1))
                        # normalize+cast on ScalarE to keep VectorE free for
                        # the PT/ctxT copies that gate the PE
                        nc.scalar.mul(
                            ctx[:, og * 512:(og + 1) * 512], ops[:],
                            rch[j][:])
                    cth[j] = ctx

                def ctxT_tr(j):
                    ctx = cth[j]
                    ctxT = ctp.tile([128, D], bf16, tag="ctxT",
                                    name=f"ctxT{j}")
                    for h in range(2):
                        tps = ps_t.tile([128, 512], bf16, tag="tp",
                                        name=f"tpc{j}_{h}")
                        for q4 in range(4):
                            dc = h * 4 + q4
                            nc.tensor.transpose(
                                tps[:, q4 * 128:(q4 + 1) * 128],
                                ctx[:, dc * 128:dc * 128 + 128],
                                ident[:])
                        nc.vector.tensor_copy(
                            ctxT[:, h * 512:(h + 1) * 512], tps[:])
                    return ctxT

                def out_mm(j, ctxT):
                    osb = op.tile([128, D], f32, tag="o", name=f"o{j}")
                    out3 = out_d.rearrange("q (og o) -> q og o", og=2)
                    for og in range(2):
                        ops = ps_m.tile([128, 512], f32, tag="av",
                                        name=f"op{j}_{og}")
                        for dc in range(DC):
                            nc.tensor.matmul(
                                ops[:],
                                ctxT[:, dc * 128:dc * 128 + 128],
                                wv[:, dc * 1024 + og * 512:dc * 1024 + og * 512 + 512],
                                start=(dc == 0), stop=(dc == DC - 1))
                        nc.vector.tensor_copy(
                            osb[:, og * 512:(og + 1) * 512], ops[:])
                        nc.sync.dma_start(
                            out3[j * 128:(j + 1) * 128, og],
                            osb[:, og * 512:(og + 1) * 512])

                # software pipeline (S(0)/S(1) already issued in B window):
                #   Ptr(0) ctx(0) | j: ctxT(j-1) Ptr(j) out(j-1) ctx(j)
                #   S(j+1) | ctxT(7) out(7)
                PT = ptr(0)
                ctx_mm(0, PT)
                for j in range(1, NT - 1):
                    cT = ctxT_tr(j - 1)
                    PT = ptr(j)
                    out_mm(j - 1, cT)
                    ctx_mm(j, PT)
                    scores_exp(j + 1, ps_s)
                # last tile: out(6) goes between ctx(7) and ctxT(7) so the
                # final norm's ScalarE latency hides under out(6)'s matmuls
                cT = ctxT_tr(NT - 2)
                PT = ptr(NT - 1)
                ctx_mm(NT - 1, PT)
                out_mm(NT - 2, cT)
                cT = ctxT_tr(NT - 1)
                out_mm(NT - 1, cT)

    nc.compile()
    _cache["nc"] = nc
    return nc


def _shard(x, Wq, Wk, Wv):
    """Build the 8 per-core input maps, pre-arranged to SBUF layouts."""
    ident = np.eye(128, dtype=np.float32).astype(BF16)
    # wq2[p, ot*1024+dc*128+oo] = Wq[ot*128+oo, dc*128+p]
    wq2 = np.ascontiguousarray(
        Wq.reshape(8, 128, 8, 128).transpose(3, 0, 2, 1)
        .reshape(128, 8192)).astype(BF16)
    # wk2[p, dc*1024+oc*128+dd] = Wk[oc*128+p, dc*128+dd]
    wk2 = np.ascontiguousarray(
        Wk.reshape(8, 128, 8, 128).transpose(1, 2, 0, 3)
        .reshape(128, 8192)).astype(BF16)
    # wv2[p, dc*1024+o] = Wv[o, dc*128+p]
    wv2 = np.ascontiguousarray(
        Wv.T.reshape(8, 128, 1024).transpose(1, 0, 2)
        .reshape(128, 8192)).astype(BF16)
    in_maps = []
    for c in range(8):
        b, p = c // 2, c % 2
        xb = np.asarray(x[b])
        # xkv2[p, dc*2048+kv] = x[kv, dc*128+p]
        xkv2 = np.ascontiguousarray(
            xb.T.reshape(8, 128, 2048).transpose(1, 0, 2)
            .reshape(128, 16384)).astype(BF16)
        # xn2[p, kb*1024+d] = x[kb*128+p, d]
        xn2 = np.ascontiguousarray(
            xb.reshape(16, 128, 1024).transpose(1, 0, 2)
            .reshape(128, 16384)).astype(BF16)
        rows = np.concatenate(
            [xb[(2 * j + p) * 128:(2 * j + p + 1) * 128, :] for j in range(8)],
            axis=0)
        # xq2[p, sg*4096+dc*512+ss] = rows[sg*512+ss, dc*128+p]
        xq2 = np.ascontiguousarray(
            rows.reshape(2, 512, 8, 128).transpose(3, 0, 2, 1)
            .reshape(128, 8192)).astype(BF16)
        masks = np.full((NT * 128, 512), NEG, np.float32)
        for j in range(NT):
            ext = _ext(j)
            ng = (ext + 3) // 4
            gw = min(512, ext * 128 - (ng - 1) * 512)
            q_abs = (2 * j + p) * 128 + np.arange(128)[:, None]
            kv_abs = (ng - 1) * 512 + np.arange(gw)[None, :]
            masks[j * 128:(j + 1) * 128, 0:gw] = np.where(
                kv_abs <= q_abs, np.float32(0), NEG)
        masks2 = np.ascontiguousarray(
            masks.reshape(8, 128, 512).transpose(1, 0, 2)
            .reshape(128, 4096)).astype(BF16)
        in_maps.append({
            "xq": xq2, "xkv": xkv2, "xn": xn2,
            "wq": wq2, "wk": wk2, "wv": wv2,
            "masks": masks2, "ident": ident,
        })
    return in_maps


def _unshard(results, dtype):
    out = np.empty((B, S, D), dtype)
    for c in range(8):
        b, p = c // 2, c % 2
        o = results[c]["out"]
        for j in range(NT):
            out[b, (2 * j + p) * 128:(2 * j + p + 1) * 128, :] = \
                o[j * 128:(j + 1) * 128, :]
    return out


def run(x, Wq, Wk, Wv, trace=False):
    from concourse.bass_utils import run_bass_kernel_spmd
    nc = _build()
    in_maps = _shard(np.asarray(x), np.asarray(Wq), np.asarray(Wk),
                     np.asarray(Wv))
    res = run_bass_kernel_spmd(nc, in_maps, core_ids=list(range(8)),
                               trace=trace)
    return _unshard(res.results, np.float32), res


def kernel(x, Wq, Wk, Wv):
    out, _ = run(x, Wq, Wk, Wv, trace=False)
    return out
